# revision 4
# baseline (speedup 1.0000x reference)
"""Fused bmm + residual kernel for Trainium2 (8 NeuronCores, data-parallel).

out[n,c,p] = x[n,c,p] + alpha * sum_q attn[n,p,q] * D[n,q,c]
  N=2048, C=512, H=W=7 (HW=49)

Sharding: batch N across 8 cores (256 each). Each core computes its slice
independently; no collectives.

Per-core scheme (channel-interleaved, pair-packed):
 - SBUF x/out tiles [128, G, 196]: partition r holds channels {4r..4r+3}
   -> 784B-contiguous DMA runs at full 128 partitions.
 - D tiles [128, G/2, 512] in "gap" layout: partition b*64+q holds
   D[pair_batch b, q, :]; rows 49:64 and 113:128 are zeroed once.
 - attn transposed on PE: in [49, 2x64-slot pair] -> out [128, 49] with
   A^T(even) at rows 0:49, A^T(odd) at rows 64:113.
 - rhs [128, 2, 49]: alpha*A^T(even) at rows 0:49 col-block 0,
   alpha*A^T(odd) at rows 64:113 col-block 1, zeros elsewhere.
 - 4 matmuls per pair (chunk j: channels c==j mod 4), K=113, M=128, N=98;
   zero rhs rows annihilate the cross-batch terms.
 - residual add on DVE straight from PSUM, store via ACT-ring DMA.
"""
import sys

sys.path.insert(0, "/opt/trn_rl_repo")

import numpy as np

# ---- static problem config (hardcoded per harness contract) ----
N_TOT, C, HW = 2048, 512, 49
N_CORES = 8
NB = N_TOT // N_CORES        # 256 batches per core
G = 16                       # batches per group (one DMA round)
NPAIR = G // 2               # pairs per group
NGROUP = NB // G             # groups per core
NBD = 4                      # rhs ring size
ND = 3                       # D-tile ring size

_cached = {}


def _build_bass():
    import concourse.bacc as bacc
    import concourse.mybir as mybir
    from concourse import tile

    f32 = mybir.dt.float32
    nc = bacc.Bacc(None, target_bir_lowering=False)

    x_d = nc.dram_tensor("x", [NB, C, HW], f32, kind="ExternalInput")
    a_d = nc.dram_tensor("attn", [NB, HW, HW], f32, kind="ExternalInput")
    d_d = nc.dram_tensor("d", [NB, HW, C], f32, kind="ExternalInput")
    al_d = nc.dram_tensor("alphac", [128, 1], f32, kind="ExternalInput")
    id_d = nc.dram_tensor("ident", [HW, HW], f32, kind="ExternalInput")
    o_d = nc.dram_tensor("out", [NB, C, HW], f32, kind="ExternalOutput")

    with tile.TileContext(nc) as tc:
        with (
            tc.tile_pool(name="const", bufs=1) as const,
            tc.tile_pool(name="bdp", bufs=NBD) as bdp,
            tc.tile_pool(name="dp", bufs=ND) as dp,
            tc.tile_pool(name="xp", bufs=3) as xp,
            tc.tile_pool(name="ap", bufs=3) as ap,
            tc.tile_pool(name="op", bufs=3) as op,
            tc.tile_pool(name="atp", bufs=3, space="PSUM") as atp,
            tc.tile_pool(name="yp", bufs=4, space="PSUM") as yp,
        ):
            ident_sb = const.tile([HW, HW], f32)
            nc.sync.dma_start(out=ident_sb, in_=id_d[:])
            alpha_sb = const.tile([128, 1], f32)
            nc.sync.dma_start(out=alpha_sb, in_=al_d[:])

            # rhs ring: zeros except the two alpha*A^T blocks written per pair
            bd_tiles = []
            for i in range(NBD):
                t = bdp.tile([128, 2, HW], f32, tag="bd")
                nc.vector.memset(t, 0.0)
                bd_tiles.append(t)

            # D-tile ring: gap rows 49:64 / 113:128 must stay finite (zero)
            d_tiles = []
            for i in range(ND):
                t = dp.tile([128, NPAIR, C], f32, tag="d")
                # zero the 32-aligned ranges covering the gap rows 49:64 and
                # 113:128; the DMA overwrites 32:49 / 96:113 with real data
                nc.vector.memset(t[32:64, :, :], 0.0)
                nc.vector.memset(t[96:128, :, :], 0.0)
                d_tiles.append(t)

            for g in range(NGROUP):
                b0 = g * G
                xs = x_d[b0:b0 + G]      # [G, C, HW]
                os_ = o_d[b0:b0 + G]
                ds = d_d[b0:b0 + G]      # [G, HW, C]
                as_ = a_d[b0:b0 + G]     # [G, HW, HW]

                x_t = xp.tile([128, G, 4 * HW], f32, tag="x")
                nc.sync.dma_start(
                    out=x_t, in_=xs.rearrange("n (r j) p -> r n (j p)", j=4)
                )
                d_t = d_tiles[g % ND]
                d_v = d_t.rearrange("(b s) i c -> b s i c", b=2)
                dsr = ds.rearrange("(i b) q c -> b q i c", b=2)
                # two plain partition-range DMAs (bases 0 and 64); they run
                # concurrently on complementary DMA-engine halves
                nc.sync.dma_start(out=d_v[0, 0:HW, :, :], in_=dsr[0])
                nc.sync.dma_start(out=d_v[1, 0:HW, :, :], in_=dsr[1])
                # attn in 64-wide slots so the pair transpose lands the odd
                # batch at PSUM rows 64:113
                a_t = ap.tile([HW, G, 64], f32, tag="a")
                nc.sync.dma_start(
                    out=a_t[:, :, 0:HW], in_=as_.rearrange("n p q -> p n q")
                )

                o_t = op.tile([128, G, 4 * HW], f32, tag="o")

                # views
                d4 = d_t.rearrange("k i (m four) -> k i four m", four=4)
                x4 = x_t.rearrange("r n (j p) -> r n j p", j=4)
                o4 = o_t.rearrange("r n (j p) -> r n j p", j=4)
                a2 = a_t.rearrange("p n q -> p (n q)")

                for i in range(NPAIR):
                    at_ps = atp.tile([128, HW], f32, tag="at")
                    # [49, 128] -> [128, 49]: rows b*64+q = A^T pair
                    nc.tensor.transpose(
                        at_ps, a2[:, 2 * i * 64:(2 * i + 2) * 64], ident_sb
                    )
                    bd = bd_tiles[i % NBD]
                    nc.vector.tensor_scalar_mul(
                        out=bd[0:HW, 0, :],
                        in0=at_ps[0:HW, :],
                        scalar1=alpha_sb[0:HW, :],
                    )
                    nc.vector.tensor_scalar_mul(
                        out=bd[64:64 + HW, 1, :],
                        in0=at_ps[64:64 + HW, :],
                        scalar1=alpha_sb[64:64 + HW, :],
                    )

                    y_ps = yp.tile([128, 4, 2 * HW], f32, tag="y")
                    bd2 = bd.rearrange("k b p -> k (b p)")
                    for j in range(4):
                        nc.tensor.matmul(
                            out=y_ps[:, j, :],
                            lhsT=d4[0:64 + HW, i, j, :],
                            rhs=bd2[0:64 + HW, :],
                            start=True,
                            stop=True,
                        )
                    # y_ps free layout: (j, b, p); regroup to (b, j, p)
                    y4 = y_ps.rearrange("r j (b p) -> r b j p", b=2)
                    nc.vector.tensor_add(
                        out=o4[:, 2 * i:2 * i + 2, :, :],
                        in0=y4,
                        in1=x4[:, 2 * i:2 * i + 2, :, :],
                    )

                nc.scalar.dma_start(
                    out=os_.rearrange("n (r j) p -> r n (j p)", j=4), in_=o_t
                )

    nc.finalize()
    return nc


def _get_nc():
    if "nc" not in _cached:
        _cached["nc"] = _build_bass()
    return _cached["nc"]


def _in_maps(x, attn, D, alpha):
    x_s = np.ascontiguousarray(x, dtype=np.float32).reshape(N_CORES, NB, C, HW)
    a_s = np.ascontiguousarray(attn, dtype=np.float32).reshape(N_CORES, NB, HW, HW)
    d_s = np.ascontiguousarray(D, dtype=np.float32).reshape(N_CORES, NB, HW, C)
    al = np.full((128, 1), np.float32(np.asarray(alpha).reshape(-1)[0]), np.float32)
    ident = np.eye(HW, dtype=np.float32)
    return [
        {"x": x_s[c], "attn": a_s[c], "d": d_s[c], "alphac": al, "ident": ident}
        for c in range(N_CORES)
    ]


def kernel(x: np.ndarray, attn: np.ndarray, D: np.ndarray, alpha: np.ndarray) -> np.ndarray:
    from concourse import bass_utils

    nc = _get_nc()
    res = bass_utils.run_bass_kernel_spmd(
        nc, _in_maps(x, attn, D, alpha), core_ids=list(range(N_CORES))
    )
    out = np.stack([res.results[c]["out"] for c in range(N_CORES)])
    return out.reshape(N_TOT, C, 7, 7).astype(np.float32, copy=False)
